# revision 8
# baseline (speedup 1.0000x reference)
"""Trainium2 Bass kernel for nn_Attention_org_45758581571643.

Reference computation (per batch b):
  x = emb[b] viewed as [S=T*N, C] (token-major)
  per head h: Q/K/V = x @ W{q,k,v}[h].T ; scores = Q K^T / sqrt(S)
  InstanceNorm over each [S,S] map, softmax over keys, ctx = probs @ V
  out = mean_h(ctx) @ Wo.T, reshaped to [B, T, C, N]

Sharding: 16 (batch, head) pairs over 8 cores -> core c handles batch c//2,
heads {2*(c%2), 2*(c%2)+1}. Head-mean and the Wo projection are linear, so each
core applies Wo to its own two-head partial sum and the host adds core pairs.

Key restructuring vs the straightforward pipeline:
- G = Wq^T Wk and V' = Wv^T Wo^T are folded on the host, so the device does
  scores = x G x^T and ctx' = probs V' directly.
- The instance-norm statistics are computed ON THE HOST in float64 from tiny
  closed forms: sum(scores) = u^T G u with u = x^T 1, and
  sum(scores^2) = <G, A G A> with A = x^T x.  Only the exp scale
  sigma = 1/sqrt(var_raw + S*eps) is shipped to the device (instance norm and
  the 1/sqrt(S) scaling reduce to a single scale; mean-centering is dropped
  entirely because softmax is shift-invariant and |sigma*scores| < ~6).
  This removes the stats barrier: exp is fused into the score matmul's
  PSUM->SBUF eviction (one ACT pass instead of copy+stats+exp).
- Softmax denominators ride along the ctx matmul for free: the moving operand
  is [V' | m] with m = 4*mask (mask zeroes the padded key rows, the 4 folds
  the mean over H=4 heads), and probs tiles are the stationary operand, so
  column 256 of each ctx PSUM tile is 4*sum_t p[t,s].

On-device layout: x lives as [C, S] (channel on partitions), scores^T as
[t, s] (keys on partitions, so denominators are partition sums). ctx comes out
as [s, d] (queries on partitions), scaled per-partition by 1/(4 den) on DVE.
S is zero-padded to 1664 = 13*128 on the key axis only; padded key rows have
V'=0 and mask=0 so they contribute nothing.
"""

import os

# Recover gracefully if a previous run left a NeuronCore wedged; must be set
# before the runtime initializes.
os.environ.setdefault("NEURON_RT_RESET_CORES", "1")

import numpy as np
from contextlib import ExitStack

B, T, C, N, H = 4, 8, 256, 196, 4
S = T * N          # 1568
SP = 1664          # 13 * 128 (padded key/seq length)
NT = SP // 128     # 13 t-tiles
SCW = 392          # s-chunk width for score tiles (4 * 392 = 1568)
NSC = S // SCW     # 4
NQC = (S + 127) // 128  # 13 query chunks for ctx (12 full + 1x32)
PAD_REAL = S - (NT - 1) * 128  # 32 real rows in the last t-tile
EPS = 1e-5

_CACHE = {}


def _build_nc(reps=1):
    import concourse.bass as bass
    import concourse.tile as tile
    from concourse import bacc, bass_isa, mybir

    f32 = mybir.dt.float32
    f32r = mybir.dt.float32r
    bf16 = mybir.dt.bfloat16
    AF = mybir.ActivationFunctionType
    ALU = mybir.AluOpType

    nc = bacc.Bacc("TRN2", target_bir_lowering=False, debug=False)

    xt_d = nc.dram_tensor("xt", [C, SP], f32r, kind="ExternalInput").ap()
    wg_d = nc.dram_tensor("wg", [2, C, C], f32r, kind="ExternalInput").ap()
    wvo_d = nc.dram_tensor("wvo", [2, C, C], f32r, kind="ExternalInput").ap()
    rs_d = nc.dram_tensor("rs", [1, 2], f32, kind="ExternalInput").ap()
    ot_d = nc.dram_tensor("ot", [S, C], f32, kind="ExternalOutput").ap()

    with tile.TileContext(nc) as tc, ExitStack() as ctx:
        xw = ctx.enter_context(tc.tile_pool(name="xw", bufs=1))
        qk = ctx.enter_context(tc.tile_pool(name="qk", bufs=1))
        vp = ctx.enter_context(tc.tile_pool(name="vp", bufs=1))
        sc = ctx.enter_context(tc.tile_pool(name="sc", bufs=1))
        cx = ctx.enter_context(tc.tile_pool(name="cx", bufs=1))
        sm = ctx.enter_context(tc.tile_pool(name="sm", bufs=4))
        pmm = ctx.enter_context(tc.tile_pool(name="pmm", bufs=3, space="PSUM"))
        pcx = ctx.enter_context(tc.tile_pool(name="pcx", bufs=3, space="PSUM"))

        # ---- load inputs (weights in first-use order; xt on two queues) ----
        wsb = {}
        for h in range(2):
            for nm, d in (("wg", wg_d), ("wvo", wvo_d)):
                for cti in range(2):
                    t = xw.tile([128, C], f32r, tag=f"{nm}{h}{cti}", name=f"{nm}{h}{cti}")
                    nc.scalar.dma_start(t[:], d[h, cti * 128:(cti + 1) * 128, :])
                    wsb[nm, h, cti] = t
        xt = [xw.tile([128, SP], f32r, tag=f"xt{i}", name=f"xt{i}") for i in range(2)]
        for kci in range(4):
            kl = slice(kci * 416, (kci + 1) * 416)
            for cti in range(2):
                eng = nc.sync if cti == 0 else nc.gpsimd
                eng.dma_start(xt[cti][:, kl],
                              xt_d[cti * 128:(cti + 1) * 128, kl])
        rs_sb = xw.tile([1, 2], f32, tag="rs_sb", name="rs_sb")
        nc.scalar.dma_start(rs_sb[:], rs_d[:, :])
        rs_b = xw.tile([128, 2], f32, tag="rs_b", name="rs_b")
        nc.gpsimd.partition_broadcast(rs_b[:], rs_sb[0:1, :])

        D = {}  # (r, h) -> {"gt": [...], "vm": [...], "st": [...]} ; r -> ctxs

        def head_qkv(r, h):
            # gt[c, s] = (G^T x^T)[c, s] so scores^T = (x^T)^T gt chunkwise
            gt = [qk.tile([128, S], f32r, tag=f"gt{i}", name=f"gt{i}", bufs=2)
                  for i in range(2)]
            for dti in range(2):
                for sci in range(NSC):
                    sl = slice(sci * SCW, (sci + 1) * SCW)
                    ps = pmm.tile([128, SCW], f32, tag="ps", name="ps")
                    for cti in range(2):
                        nc.tensor.matmul(
                            ps[:], wsb["wg", h, cti][:, dti * 128:(dti + 1) * 128],
                            xt[cti][:, sl], start=(cti == 0), stop=(cti == 1))
                    nc.vector.tensor_copy(gt[dti][:, sl], ps[:])
            # vm[t, 0:256] = (x V')[t, :]; vm[:, 256] = 4*mask (den column)
            vm = [vp.tile([128, C + 1], bf16, tag=f"vm{i}", name=f"vm{i}", bufs=2)
                  for i in range(NT)]
            for ti in range(NT):
                tsl = slice(ti * 128, (ti + 1) * 128)
                ps = pmm.tile([128, SCW], f32, tag="ps", name="ps")
                for cti in range(2):
                    nc.tensor.matmul(
                        ps[:, 0:C], xt[cti][:, tsl], wsb["wvo", h, cti][:],
                        start=(cti == 0), stop=(cti == 1))
                nc.vector.tensor_copy(vm[ti][:, 0:C], ps[:, 0:C])
                if ti == NT - 1:
                    # base-partition alignment forbids a [32:128] memset span;
                    # zero the whole column, then set the 32 real rows
                    nc.vector.memset(vm[ti][:, C:C + 1], 0.0)
                    nc.vector.memset(vm[ti][0:PAD_REAL, C:C + 1], float(H))
                else:
                    nc.vector.memset(vm[ti][:, C:C + 1], float(H))
            D[r, h] = {"gt": gt, "vm": vm}

        def head_scores(r, h):
            # st[t, s] = exp(sigma_h * scores[s, t]), fused PSUM->SBUF in bf16
            gt = D[r, h]["gt"]
            st = [sc.tile([128, S], bf16, tag=f"st{i}", name=f"st{i}", bufs=2)
                  for i in range(NT)]
            # sci-major: each 392-wide query band completes across all key
            # tiles before the next band starts, so ctx chunks (which read one
            # 128-query stripe of every tile) become ready progressively
            # instead of all waiting on the final tile's exp.
            for sci in range(NSC):
                sl = slice(sci * SCW, (sci + 1) * SCW)
                for ti in range(NT):
                    tsl = slice(ti * 128, (ti + 1) * 128)
                    ps = pmm.tile([128, SCW], f32, tag="ps", name="ps")
                    for cti in range(2):
                        nc.tensor.matmul(
                            ps[:], xt[cti][:, tsl], gt[cti][:, sl],
                            start=(cti == 0), stop=(cti == 1))
                    nc.scalar.activation(out=st[ti][:, sl], in_=ps[:], func=AF.Exp,
                                         scale=rs_b[:, h:h + 1])
            D[r, h]["st"] = st

        def head_ctx(r, h):
            # ctx[s, d] += p^T V' / (4 den[s]) ; den rides in column 256
            st = D[r, h]["st"]
            vm = D[r, h]["vm"]
            if h == 0:
                D[r] = [cx.tile([128, C], f32, tag=f"ctx{i}", name=f"ctx{i}",
                                bufs=1) for i in range(NQC)]
            for ci in range(NQC):
                m = min(128, S - ci * 128)
                cs = ci * 128
                ps = pcx.tile([128, C + 1], f32, tag="psx", name="psx")
                for ti in range(NT):
                    nc.tensor.matmul(ps[0:m, :], st[ti][:, cs:cs + m], vm[ti][:],
                                     start=(ti == 0), stop=(ti == NT - 1))
                rec = sm.tile([128, 1], f32, tag="rec", name="rec", bufs=3)
                nc.vector.reciprocal_approx_fast(out=rec[0:m], in_=ps[0:m, C:C + 1])
                ctxs = D[r][ci]
                if h == 0:
                    nc.vector.tensor_scalar_mul(ctxs[0:m, :], ps[0:m, 0:C], rec[0:m])
                else:
                    nc.vector.scalar_tensor_tensor(
                        out=ctxs[0:m, :], in0=ps[0:m, 0:C], scalar=rec[0:m],
                        in1=ctxs[0:m, :], op0=ALU.mult, op1=ALU.add)
                    nc.sync.dma_start(ot_d[cs:cs + m, :], ctxs[0:m, :])

        # software pipeline: the next head's projections+scores are emitted
        # between this head's scores and ctx so PE has work while ACT drains
        # the exp tail (exp is slower than the score matmuls it chases).
        for r in range(reps):
            if r == 0:
                head_qkv(0, 0)
                head_scores(0, 0)
                head_qkv(0, 1)
            head_ctx(r, 0)
            head_scores(r, 1)
            if r + 1 < reps:
                head_qkv(r + 1, 0)
            head_ctx(r, 1)
            if r + 1 < reps:
                head_scores(r + 1, 0)
                head_qkv(r + 1, 1)
            D.pop((r, 0), None)
            D.pop((r, 1), None)
            D.pop(r, None)

    nc.finalize()
    return nc


def _get_nc(reps=1):
    key = ("nc", reps)
    if key not in _CACHE:
        _CACHE[key] = _build_nc(reps)
    return _CACHE[key]


def make_in_maps(emb, Wq, Wk, Wv, Wo):
    emb = np.ascontiguousarray(emb, dtype=np.float32)
    Wq = np.asarray(Wq, np.float64)
    Wk = np.asarray(Wk, np.float64)
    Wv = np.asarray(Wv, np.float64)
    Wo = np.asarray(Wo, np.float64)
    # wg[h] = Wq[h]^T @ Wk[h]  (scores = x wg^T x^T per head, see kernel docstring)
    wg = np.einsum("hdc,hde->hce", Wq, Wk)
    # wvo[h] = Wv[h]^T @ Wo^T  (folds the output projection into V)
    wvo = np.einsum("hdc,ed->hce", Wv, Wo).astype(np.float32)
    wg32 = wg.astype(np.float32)
    # closed-form instance-norm stats per (batch, head):
    #   sum(scores)  = u^T G u   (u = column sums of x)
    #   sum(scores^2)= <G, A G A> (A = x^T x)
    # exp scale folds /sqrt(S) and rsqrt(var + eps) into one scalar.
    sigma = np.empty((B, H), np.float32)
    xts = []
    for b in range(B):
        xt = np.zeros((C, SP), np.float32)
        xt[:, :S] = emb[b].transpose(1, 0, 2).reshape(C, S)
        xts.append(xt)
        xb = xt[:, :S].astype(np.float64)
        A = xb @ xb.T
        u = xb.sum(axis=1)
        for h in range(H):
            G = wg[h]
            m_raw = u @ G @ u / (S * S)
            ssq = float(np.sum(G * (A @ G @ A)))
            var_raw = ssq / (S * S) - m_raw * m_raw
            sigma[b, h] = 1.0 / np.sqrt(var_raw + S * EPS)
    in_maps = []
    for core in range(8):
        b, g = core // 2, core % 2
        hs = [2 * g, 2 * g + 1]
        in_maps.append({
            "xt": xts[b],
            "wg": np.ascontiguousarray(wg32[hs]),
            "wvo": np.ascontiguousarray(wvo[hs]),
            "rs": sigma[b, hs].reshape(1, 2),
        })
    return in_maps


def gather_out(results):
    out = np.empty((B, S, C), np.float32)
    for b in range(B):
        out[b] = results[2 * b]["ot"] + results[2 * b + 1]["ot"]
    return out.reshape(B, T, C, N)


def _get_runner():
    """Cached PJRT executable: run_bass_kernel_spmd re-jits per call, which
    costs seconds of XLA compile on every invocation; build the sharded
    callable once and reuse it."""
    if "runner" in _CACHE:
        return _CACHE["runner"]
    import jax
    from jax.sharding import Mesh, PartitionSpec, NamedSharding
    from jax.experimental.shard_map import shard_map
    from concourse import mybir
    from concourse.bass2jax import (_bass_exec_p, install_neuronx_cc_hook,
                                    partition_id_tensor)

    install_neuronx_cc_hook()
    nc = _get_nc()
    in_names, out_names, out_avals, zero_shapes = [], [], [], []
    partition_name = nc.partition_id_tensor.name if nc.partition_id_tensor else None
    for alloc in nc.m.functions[0].allocations:
        if not isinstance(alloc, mybir.MemoryLocationSet):
            continue
        name = alloc.memorylocations[0].name
        if alloc.kind == "ExternalInput":
            if name != partition_name:
                in_names.append(name)
        elif alloc.kind == "ExternalOutput":
            shape = tuple(alloc.tensor_shape)
            dtype = mybir.dt.np(alloc.dtype)
            out_names.append(name)
            out_avals.append(jax.core.ShapedArray(shape, dtype))
            zero_shapes.append((shape, dtype))
    n_params = len(in_names)
    all_in = list(in_names) + list(out_names)
    if partition_name is not None:
        all_in.append(partition_name)

    def _body(*args):
        operands = list(args)
        if partition_name is not None:
            operands.append(partition_id_tensor())
        return tuple(_bass_exec_p.bind(
            *operands, out_avals=tuple(out_avals), in_names=tuple(all_in),
            out_names=tuple(out_names), lowering_input_output_aliases=(),
            sim_require_finite=True, sim_require_nnan=True, nc=nc))

    n_cores = 8
    mesh = Mesh(np.asarray(jax.devices()[:n_cores]), ("core",))
    sharded = jax.jit(
        shard_map(_body, mesh=mesh,
                  in_specs=(PartitionSpec("core"),) * (n_params + len(out_names)),
                  out_specs=(PartitionSpec("core"),) * len(out_names),
                  check_rep=False),
        keep_unused=True)

    def run(in_maps):
        per_core = [[np.asarray(m[nm]) for nm in in_names] for m in in_maps]
        concat_in = [np.concatenate([per_core[c][i] for c in range(n_cores)], axis=0)
                     for i in range(n_params)]
        concat_zeros = [np.zeros((n_cores * s[0], *s[1:]), d)
                        for (s, d) in zero_shapes]
        outs = sharded(*concat_in, *concat_zeros)
        return [{out_names[i]: np.asarray(outs[i]).reshape(
                     n_cores, *out_avals[i].shape)[c]
                 for i in range(len(out_names))} for c in range(n_cores)]

    _CACHE["runner"] = run
    return run


def kernel(emb, Wq, Wk, Wv, Wo):
    in_maps = make_in_maps(emb, Wq, Wk, Wv, Wo)
    try:
        return gather_out(_get_runner()(in_maps))
    except Exception:
        from concourse.bass_utils import run_bass_kernel_spmd
        nc = _get_nc()
        res = run_bass_kernel_spmd(nc, in_maps, list(range(8)))
        return gather_out(res.results)
